# revision 10
# baseline (speedup 1.0000x reference)
"""DistanceSVM forward on 8 TRN2 NeuronCores.

out[n] = max_avg_distance - sum_c w_c * ||x_n - center_c||,
w = |coefs| / sum(|coefs|)   (unnormalized if the sum is 0).

Strategy (data-parallel over N, centers/coefs replicated, per spec hint):
  - Fold the whole distance computation into one augmented GEMM:
        2^S * w_c^2 * d2[n,c] =
            [x_n, x2hi_n, x2lo_n, 1] . [-2*u_c*center_c ; u_c ; u_c ; u_c*c2_c]
    with u_c = 2^S * w_c^2 >= 0 (S rescales u into fp16-friendly range),
    so  w_c * d[n,c] = sqrt(2^-S * psum).  d2 >= ~24 for randn data in
    64-d, so no relu is needed before sqrt.  x2 is carried as an fp16
    hi/lo pair to keep the large self-term at ~fp32 accuracy.
  - TensorE (fp16 operands, fp32 PSUM accumulate, 1 cycle/row) computes
    the augmented GEMM: 4 x [128, 512] matmuls per [128, 2048] PSUM group
    (two 128-row n-tiles per group).
  - ScalarE applies Sqrt (with the free 2^-S prescale) in one [128, 2048]
    instruction per group, in place in PSUM.
  - VectorE folds each n-tile's two 512-wide halves with a fused
    tensor_tensor_reduce (add + row-sum) -> weighted average per row.
  - Final (128, TILES) epilogue: out = mad - wavg, then one contiguous DMA.
  - Host pre/post (numpy, O(N*D)): builds the transposed augmented fp16
    operands, reassembles the sharded output.
"""

import numpy as np

import concourse.bacc as bacc
import concourse.bass as bass
import concourse.mybir as mybir
import concourse.tile as tile
from concourse.bass_utils import run_bass_kernel_spmd

N_CORES = 8
N, C, D = 131072, 1024, 64
NS = N // N_CORES            # rows per core
P = 128                      # partitions
TILES = NS // P              # n-tiles per core (128)
K = D + 3                    # x, x2_hi, x2_lo, ones
S = 22                       # global exponent scale on u = w^2
NCHUNK = 16                  # input DMA chunks

_nc_cache = None


def _build_nc():
    f32 = mybir.dt.float32
    f16 = mybir.dt.float16
    nc = bacc.Bacc("TRN2", target_bir_lowering=False)
    xaT = nc.dram_tensor("xaT", [K, NS], f16, kind="ExternalInput")
    cw = nc.dram_tensor("cw", [K, C], f16, kind="ExternalInput")
    mad = nc.dram_tensor("mad", [1], f32, kind="ExternalInput")
    out = nc.dram_tensor("out", [P, TILES], f32, kind="ExternalOutput")

    with tile.TileContext(nc) as tc:
        with tc.tile_pool(name="xp", bufs=1) as xp, \
             tc.tile_pool(name="singles", bufs=1) as singles, \
             tc.tile_pool(name="acc", bufs=1) as accp, \
             tc.tile_pool(name="sq", bufs=2) as sqp, \
             tc.tile_pool(name="ps", bufs=2, space="PSUM") as psp:
            # cen halves first (MM of c-chunk 0 only needs the first half);
            # x chunks ramp up in size so the first matmul starts ASAP, and
            # alternate between the sync and gpsimd DMA queues so descriptor
            # generation isn't serialized on one sequencer.
            cen = singles.tile([K, C], f16, tag="cen")
            nc.sync.dma_start(out=cen[:, 0:512], in_=cw[:, 0:512])

            wd = accp.tile([P, TILES], f32, tag="wd")

            chunk_cols = [128, 128, 256, 512, 1024] + [2048] * 7
            assert sum(chunk_cols) == NS
            xs = []          # (tile, start_col) per chunk
            col = 0
            for kk, cc in enumerate(chunk_cols):
                xt = xp.tile([K, cc], f16, tag=f"x{kk}")
                eng = nc.sync if kk % 2 == 0 else nc.gpsimd
                eng.dma_start(out=xt, in_=xaT[:, col:col + cc])
                xs.append((xt, col))
                col += cc
            nc.sync.dma_start(out=cen[:, 512:1024], in_=cw[:, 512:1024])
            mad_sb = singles.tile([P, 1], f32, tag="mad")
            nc.sync.dma_start(out=mad_sb, in_=mad[:].to_broadcast((P, 1)))

            def lhsT_for(t):
                n0 = t * P
                for xt, c0 in xs:
                    if c0 <= n0 < c0 + xt.shape[1]:
                        return xt[:, n0 - c0:n0 - c0 + P]
                raise AssertionError(t)
            add = mybir.AluOpType.add
            sqrt_fn = mybir.ActivationFunctionType.Sqrt
            inv_scale = float(2.0 ** (-S))
            for g in range(TILES // 2):
                ps = psp.tile([P, 2048], f32, tag="ps")
                for h in range(2):
                    t = 2 * g + h
                    lhsT = lhsT_for(t)
                    base = h * 1024
                    nc.tensor.matmul(ps[:, base:base + 512], lhsT=lhsT,
                                     rhs=cen[:, 0:512], start=True, stop=True)
                    nc.tensor.matmul(ps[:, base + 512:base + 1024], lhsT=lhsT,
                                     rhs=cen[:, 512:1024], start=True, stop=True)
                # One wide sqrt on ACT; per-tile halves-fold + row-sum on DVE
                # via scalar_tensor_tensor's fused accumulator.
                sq = sqp.tile([P, 2048], f32, tag="sq")
                nc.scalar.activation(sq, ps, sqrt_fn, scale=inv_scale)
                for h in range(2):
                    t = 2 * g + h
                    base = h * 1024
                    dummy = sqp.tile([P, 512], f32, tag="dm")
                    nc.vector.scalar_tensor_tensor(
                        out=dummy, in0=sq[:, base:base + 512], scalar=0.0,
                        in1=sq[:, base + 512:base + 1024],
                        op0=add, op1=add, accum_out=wd[:, t:t + 1])

            out_sb = accp.tile([P, TILES], f32, tag="os")
            nc.vector.tensor_scalar(out=out_sb, in0=wd,
                                    scalar1=-1.0, scalar2=mad_sb,
                                    op0=mybir.AluOpType.mult,
                                    op1=mybir.AluOpType.add)
            nc.sync.dma_start(out=out[:, :], in_=out_sb)
    nc.finalize()
    return nc


def _get_nc():
    global _nc_cache
    if _nc_cache is None:
        _nc_cache = _build_nc()
    return _nc_cache


def build_in_maps(inputs, centers, coefs, max_avg_distance):
    x = np.ascontiguousarray(np.asarray(inputs, dtype=np.float32).reshape(N, D))
    cen = np.asarray(centers, dtype=np.float32)
    co = np.asarray(coefs, dtype=np.float32)
    mad = np.asarray(max_avg_distance, dtype=np.float32).reshape(1)

    w = np.abs(co)
    s = np.float32(w.sum(dtype=np.float32))
    if s != 0.0:
        w = (w / s).astype(np.float32)
    u = (w.astype(np.float64) ** 2) * (2.0 ** S)
    c2 = (cen.astype(np.float64) ** 2).sum(axis=1)

    cw = np.empty((K, C), dtype=np.float16)
    cw[:D] = (-2.0 * u[:, None] * cen.astype(np.float64)).T.astype(np.float16)
    cw[D] = u.astype(np.float16)
    cw[D + 1] = cw[D]
    cw[D + 2] = (u * c2).astype(np.float16)

    in_maps = []
    for g in range(N_CORES):
        xg = x[g * NS:(g + 1) * NS]
        x2 = (xg.astype(np.float64) ** 2).sum(axis=1)
        x2_hi = x2.astype(np.float16)
        x2_lo = (x2 - x2_hi.astype(np.float64)).astype(np.float16)
        xaT = np.empty((K, NS), dtype=np.float16)
        xaT[:D] = xg.T.astype(np.float16)
        xaT[D] = x2_hi
        xaT[D + 1] = x2_lo
        xaT[D + 2] = 1.0
        in_maps.append({"xaT": xaT, "cw": cw, "mad": mad})
    return in_maps


def kernel(inputs, centers, coefs, max_avg_distance):
    in_maps = build_in_maps(inputs, centers, coefs, max_avg_distance)
    res = run_bass_kernel_spmd(_get_nc(), in_maps, core_ids=list(range(N_CORES)))
    full = np.concatenate(
        [np.asarray(res.results[g]["out"]).T.reshape(-1) for g in range(N_CORES)]
    )
    return full.astype(np.float32)


# revision 11
# speedup vs baseline: 1.2748x; 1.2748x over previous
"""DistanceSVM forward on 8 TRN2 NeuronCores.

out[n] = max_avg_distance - sum_c w_c * ||x_n - center_c||,
w = |coefs| / sum(|coefs|)   (unnormalized if the sum is 0).

Strategy (data-parallel over N, centers/coefs replicated, per spec hint):
  - Fold the whole distance computation into one augmented GEMM:
        2^S * w_c^2 * d2[n,c] =
            [x_n, x2hi_n, x2lo_n, 1] . [-2*u_c*center_c ; u_c ; u_c ; u_c*c2_c]
    with u_c = 2^S * w_c^2 >= 0 (S rescales u into fp16-friendly range),
    so  w_c * d[n,c] = sqrt(2^-S * psum).  d2 >= ~24 for randn data in
    64-d, so no relu is needed before sqrt.  x2 is carried as an fp16
    hi/lo pair to keep the large self-term at ~fp32 accuracy.
  - TensorE (fp16 operands, fp32 PSUM accumulate, 1 cycle/row) computes
    the augmented GEMM: 4 x [128, 512] matmuls per [128, 2048] PSUM group
    (two 128-row n-tiles per group).
  - ScalarE applies Sqrt (with the free 2^-S prescale) in one [128, 2048]
    instruction per group, in place in PSUM.
  - VectorE folds each n-tile's two 512-wide halves with a fused
    tensor_tensor_reduce (add + row-sum) -> weighted average per row.
  - Final (128, TILES) epilogue: out = mad - wavg, then one contiguous DMA.
  - Host pre/post (numpy, O(N*D)): builds the transposed augmented fp16
    operands, reassembles the sharded output.
"""

import numpy as np

import concourse.bacc as bacc
import concourse.bass as bass
import concourse.mybir as mybir
import concourse.tile as tile
from concourse.bass_utils import run_bass_kernel_spmd

N_CORES = 8
N, C, D = 131072, 1024, 64
NS = N // N_CORES            # rows per core
P = 128                      # partitions
TILES = NS // P              # n-tiles per core (128)
K = D + 3                    # x, x2_hi, x2_lo, ones
S = 22                       # global exponent scale on u = w^2
NCHUNK = 16                  # input DMA chunks

_nc_cache = None


def _build_nc():
    f32 = mybir.dt.float32
    f16 = mybir.dt.float16
    nc = bacc.Bacc("TRN2", target_bir_lowering=False)
    xaT = nc.dram_tensor("xaT", [K, NS], f16, kind="ExternalInput")
    cw = nc.dram_tensor("cw", [K, C], f16, kind="ExternalInput")
    mad = nc.dram_tensor("mad", [1], f32, kind="ExternalInput")
    out = nc.dram_tensor("out", [P, TILES], f32, kind="ExternalOutput")

    with tile.TileContext(nc) as tc:
        with tc.tile_pool(name="xp", bufs=1) as xp, \
             tc.tile_pool(name="singles", bufs=1) as singles, \
             tc.tile_pool(name="acc", bufs=1) as accp, \
             tc.tile_pool(name="sq", bufs=2) as sqp, \
             tc.tile_pool(name="ps", bufs=2, space="PSUM") as psp:
            # cen halves first (MM of c-chunk 0 only needs the first half);
            # x chunks ramp up in size so the first matmul starts ASAP, and
            # alternate between the sync and gpsimd DMA queues so descriptor
            # generation isn't serialized on one sequencer.
            cen = singles.tile([K, C], f16, tag="cen")
            nc.sync.dma_start(out=cen[:, 0:512], in_=cw[:, 0:512])
            nc.sync.dma_start(out=cen[:, 512:1024], in_=cw[:, 512:1024])
            mad_sb = singles.tile([P, 1], f32, tag="mad")
            nc.sync.dma_start(out=mad_sb, in_=mad[:].to_broadcast((P, 1)))

            wd = accp.tile([P, TILES], f32, tag="wd")

            chunk_cols = [128, 128, 256, 512, 1024] + [2048] * 7
            assert sum(chunk_cols) == NS
            xs = []          # (tile, start_col) per chunk
            col = 0
            for kk, cc in enumerate(chunk_cols):
                xt = xp.tile([K, cc], f16, tag=f"x{kk}")
                nc.gpsimd.dma_start(out=xt, in_=xaT[:, col:col + cc])
                xs.append((xt, col))
                col += cc

            def lhsT_for(t):
                n0 = t * P
                for xt, c0 in xs:
                    if c0 <= n0 < c0 + xt.shape[1]:
                        return xt[:, n0 - c0:n0 - c0 + P]
                raise AssertionError(t)
            add = mybir.AluOpType.add
            sqrt_fn = mybir.ActivationFunctionType.Sqrt
            inv_scale = float(2.0 ** (-S))
            for g in range(TILES // 2):
                ps = psp.tile([P, 2048], f32, tag="ps")
                for h in range(2):
                    t = 2 * g + h
                    lhsT = lhsT_for(t)
                    base = h * 1024
                    nc.tensor.matmul(ps[:, base:base + 512], lhsT=lhsT,
                                     rhs=cen[:, 0:512], start=True, stop=True)
                    nc.tensor.matmul(ps[:, base + 512:base + 1024], lhsT=lhsT,
                                     rhs=cen[:, 512:1024], start=True, stop=True)
                # One wide sqrt on ACT; per-tile halves-fold + row-sum on DVE
                # via scalar_tensor_tensor's fused accumulator.
                sq = sqp.tile([P, 2048], f32, tag="sq")
                nc.scalar.activation(sq, ps, sqrt_fn, scale=inv_scale)
                for h in range(2):
                    t = 2 * g + h
                    base = h * 1024
                    dummy = sqp.tile([P, 512], f32, tag="dm")
                    nc.vector.scalar_tensor_tensor(
                        out=dummy, in0=sq[:, base:base + 512], scalar=0.0,
                        in1=sq[:, base + 512:base + 1024],
                        op0=add, op1=add, accum_out=wd[:, t:t + 1])

            out_sb = accp.tile([P, TILES], f32, tag="os")
            nc.vector.tensor_scalar(out=out_sb, in0=wd,
                                    scalar1=-1.0, scalar2=mad_sb,
                                    op0=mybir.AluOpType.mult,
                                    op1=mybir.AluOpType.add)
            nc.sync.dma_start(out=out[:, :], in_=out_sb)
    nc.finalize()
    return nc


def _get_nc():
    global _nc_cache
    if _nc_cache is None:
        _nc_cache = _build_nc()
    return _nc_cache


def build_in_maps(inputs, centers, coefs, max_avg_distance):
    x = np.ascontiguousarray(np.asarray(inputs, dtype=np.float32).reshape(N, D))
    cen = np.asarray(centers, dtype=np.float32)
    co = np.asarray(coefs, dtype=np.float32)
    mad = np.asarray(max_avg_distance, dtype=np.float32).reshape(1)

    w = np.abs(co)
    s = np.float32(w.sum(dtype=np.float32))
    if s != 0.0:
        w = (w / s).astype(np.float32)
    u = (w.astype(np.float64) ** 2) * (2.0 ** S)
    c2 = (cen.astype(np.float64) ** 2).sum(axis=1)

    cw = np.empty((K, C), dtype=np.float16)
    cw[:D] = (-2.0 * u[:, None] * cen.astype(np.float64)).T.astype(np.float16)
    cw[D] = u.astype(np.float16)
    cw[D + 1] = cw[D]
    cw[D + 2] = (u * c2).astype(np.float16)

    in_maps = []
    for g in range(N_CORES):
        xg = x[g * NS:(g + 1) * NS]
        x2 = (xg.astype(np.float64) ** 2).sum(axis=1)
        x2_hi = x2.astype(np.float16)
        x2_lo = (x2 - x2_hi.astype(np.float64)).astype(np.float16)
        xaT = np.empty((K, NS), dtype=np.float16)
        xaT[:D] = xg.T.astype(np.float16)
        xaT[D] = x2_hi
        xaT[D + 1] = x2_lo
        xaT[D + 2] = 1.0
        in_maps.append({"xaT": xaT, "cw": cw, "mad": mad})
    return in_maps


def kernel(inputs, centers, coefs, max_avg_distance):
    in_maps = build_in_maps(inputs, centers, coefs, max_avg_distance)
    res = run_bass_kernel_spmd(_get_nc(), in_maps, core_ids=list(range(N_CORES)))
    full = np.concatenate(
        [np.asarray(res.results[g]["out"]).T.reshape(-1) for g in range(N_CORES)]
    )
    return full.astype(np.float32)
